# revision 4
# baseline (speedup 1.0000x reference)
"""Dot-product attention (B=32, S=2048, D=1024) on 8 TRN2 NeuronCores.

Data-parallel over batch: each core gets B_local=4 batches. Per batch the
full K slab (S x D = 8 MiB) is streamed HBM->SBUF exactly once:
  - energies  e[s] = sum_d K[s,d]*q[d]   via DVE tensor_tensor_reduce
    (K tile [128s, 1024d] * partition-replicated q, fused free-dim reduce)
  - softmax over all S=2048 energies (per-partition max/sum + PE-transpose
    cross-partition reduction, ACT exp with fused row-sum accumulation)
  - values    v[d] = sum_s p[s]*K[s,d]   via PE matmuls (p column as lhsT,
    resident K tiles as rhs, PSUM accumulation over s-tiles)
HBM traffic per core ~= 32 MiB read once -> memory-roofline bound.
"""

import sys

if "/opt/trn_rl_repo" not in sys.path:
    sys.path.insert(0, "/opt/trn_rl_repo")

from contextlib import ExitStack

import numpy as np

import concourse.bacc as bacc
import concourse.bass as bass
import concourse.tile as tile
from concourse import mybir
from concourse.masks import make_identity

N_CORES = 8
S, B, D = 2048, 32, 1024
BL = B // N_CORES          # batches per core
P = 128                    # s-tile rows (SBUF partitions)
NST = S // P               # s-tiles per batch
F32 = mybir.dt.float32


def build_attention_kernel(s=S, bl=BL, d=D, k_bufs_batches=2):
    """Build + compile the per-core Bass program. Returns the Bacc object."""
    nst = s // P
    nc = bacc.Bacc(
        "TRN2", target_bir_lowering=False, debug=False, num_devices=N_CORES
    )
    enc = nc.dram_tensor(
        "encoder_outputs", [s, bl, d], F32, kind="ExternalInput"
    ).ap()
    dec = nc.dram_tensor(
        "decoder_hidden", [1, bl, d], F32, kind="ExternalInput"
    ).ap()
    vals = nc.dram_tensor("attn_values", [bl, d], F32, kind="ExternalOutput").ap()
    scor = nc.dram_tensor("attn_scores", [bl, s], F32, kind="ExternalOutput").ap()

    with tile.TileContext(nc) as tc, ExitStack() as ctx:
        _attention_body(ctx, tc, enc, dec, vals, scor, s, bl, d, nst, k_bufs_batches)

    nc.compile()
    return nc


def _attention_body(ctx, tc, enc, dec, vals, scor, s, bl, d, nst, k_bufs_batches):
    nc = tc.nc
    AF = mybir.ActivationFunctionType
    ALU = mybir.AluOpType
    AX = mybir.AxisListType

    const_pool = ctx.enter_context(tc.tile_pool(name="const", bufs=1))
    qrep_pool = ctx.enter_context(tc.tile_pool(name="qrep", bufs=1))
    kpool = ctx.enter_context(tc.tile_pool(name="k", bufs=k_bufs_batches * nst))
    prod_pool = ctx.enter_context(tc.tile_pool(name="prod", bufs=3))
    e_pool = ctx.enter_context(tc.tile_pool(name="e", bufs=bl))
    p_pool = ctx.enter_context(tc.tile_pool(name="p", bufs=bl))
    small_pool = ctx.enter_context(tc.tile_pool(name="small", bufs=12))
    out_pool = ctx.enter_context(tc.tile_pool(name="outs", bufs=4))

    tp_psum = ctx.enter_context(tc.tile_pool(name="tp_psum", bufs=2, space="PSUM"))
    bc_psum = ctx.enter_context(tc.tile_pool(name="bc_psum", bufs=2, space="PSUM"))
    sc_psum = ctx.enter_context(tc.tile_pool(name="sc_psum", bufs=1, space="PSUM"))
    v_psum = ctx.enter_context(tc.tile_pool(name="v_psum", bufs=1, space="PSUM"))

    identity = const_pool.tile([P, P], F32)
    make_identity(nc, identity)
    ones_row = const_pool.tile([1, P], F32)
    nc.vector.memset(ones_row[:], 1.0)
    neg_ones_row = const_pool.tile([1, P], F32)
    nc.vector.memset(neg_ones_row[:], -1.0)

    # Replicate q for all local batches across the 128 partitions.
    q_flat = const_pool.tile([1, bl, d], F32)
    nc.sync.dma_start(q_flat[:], dec[0:1])
    qrep = qrep_pool.tile([P, bl, d], F32)
    nc.gpsimd.partition_broadcast(qrep[:], q_flat[:])

    for b in range(bl):
        # ---- phase 1: stream K, compute energies ----
        E = e_pool.tile([P, nst], F32)
        k_tiles = []
        for st in range(nst):
            kt = kpool.tile([P, d], F32)
            nc.sync.dma_start(kt[:], enc[st * P : (st + 1) * P, b])
            prod = prod_pool.tile([P, d], F32)
            nc.vector.tensor_mul(prod[:], kt[:], qrep[:, b])
            red = prod_pool.tile([P, d], F32, tag="red")
            nc.scalar.activation(
                red[:], prod[:], AF.Copy, accum_out=E[:, st : st + 1]
            )
            k_tiles.append(kt)

        # ---- softmax over all s (one reference row) ----
        rowmax = small_pool.tile([P, 1], F32)
        nc.vector.reduce_max(rowmax[:], E[:], axis=AX.X)
        tp = tp_psum.tile([1, P], F32, tag="tp")
        nc.tensor.transpose(tp[:], rowmax[:], identity[:])
        gmax = small_pool.tile([1, 1], F32)
        nc.vector.reduce_max(gmax[:], tp[:], axis=AX.X)
        negmax_ps = bc_psum.tile([P, 1], F32, tag="bc")
        nc.tensor.matmul(negmax_ps[:], neg_ones_row[:], gmax[:], start=True, stop=True)
        negmax = small_pool.tile([P, 1], F32)
        nc.scalar.copy(negmax[:], negmax_ps[:])

        Pm = p_pool.tile([P, nst], F32)
        lrow = small_pool.tile([P, 1], F32)
        nc.scalar.activation(
            Pm[:], E[:], AF.Exp, bias=negmax[:], accum_out=lrow[:]
        )
        tp2 = tp_psum.tile([1, P], F32, tag="tp")
        nc.tensor.transpose(tp2[:], lrow[:], identity[:])
        lsum = small_pool.tile([1, 1], F32)
        nc.vector.reduce_sum(lsum[:], tp2[:], axis=AX.X)
        invl = small_pool.tile([1, 1], F32)
        nc.vector.reciprocal(invl[:], lsum[:])
        invl_ps = bc_psum.tile([P, 1], F32, tag="bc")
        nc.tensor.matmul(invl_ps[:], ones_row[:], invl[:], start=True, stop=True)
        invl_bc = small_pool.tile([P, 1], F32)
        nc.scalar.copy(invl_bc[:], invl_ps[:])
        nc.vector.tensor_scalar_mul(Pm[:], Pm[:], invl_bc[:])

        # ---- scores out: [128s, nst] -> [nst, 128] -> HBM row b ----
        sps = sc_psum.tile([nst, P], F32)
        nc.tensor.transpose(sps[:], Pm[:], identity[:])
        s_sb = out_pool.tile([nst, P], F32)
        nc.scalar.copy(s_sb[:], sps[:])
        nc.sync.dma_start(
            scor[b : b + 1].rearrange("o (p f) -> (o p) f", p=nst), s_sb[:]
        )

        # ---- phase 2: values = p^T K, accumulated over s-tiles ----
        vps = v_psum.tile([1, d], F32)
        for h in range(d // 512):
            for st in range(nst):
                nc.tensor.matmul(
                    vps[:, h * 512 : (h + 1) * 512],
                    Pm[:, st : st + 1],
                    k_tiles[st][:, h * 512 : (h + 1) * 512],
                    start=(st == 0),
                    stop=(st == nst - 1),
                )
        v_sb = out_pool.tile([1, d], F32)
        nc.scalar.copy(v_sb[:], vps[:])
        nc.sync.dma_start(vals[b : b + 1], v_sb[:])


_NC_CACHE = None


def _get_nc():
    global _NC_CACHE
    if _NC_CACHE is None:
        _NC_CACHE = build_attention_kernel()
    return _NC_CACHE


def kernel(decoder_hidden, encoder_outputs, _trace=False, _tmpdir=None):
    from concourse.bass_utils import run_bass_kernel_spmd

    decoder_hidden = np.asarray(decoder_hidden, dtype=np.float32)
    encoder_outputs = np.asarray(encoder_outputs, dtype=np.float32)
    nc = _get_nc()
    in_maps = []
    for c in range(N_CORES):
        sl = slice(c * BL, (c + 1) * BL)
        in_maps.append(
            {
                "encoder_outputs": np.ascontiguousarray(encoder_outputs[:, sl, :]),
                "decoder_hidden": np.ascontiguousarray(decoder_hidden[:, sl, :]),
            }
        )
    res = run_bass_kernel_spmd(
        nc, in_maps, list(range(N_CORES)), trace=_trace, tmpdir=_tmpdir
    )
    values = np.concatenate(
        [res.results[c]["attn_values"] for c in range(N_CORES)], axis=0
    )
    scores = np.concatenate(
        [res.results[c]["attn_scores"] for c in range(N_CORES)], axis=0
    )
    if _trace:
        return (values, scores), res
    return (values, scores)
